# revision 14
# baseline (speedup 1.0000x reference)
"""Trainium2 Bass kernel for a 2-layer multi-head GAT (N=4096, 8 heads).

Self-contained: call kernel(**inputs) with the full (unsharded) inputs from
setup_inputs(); returns (result1, result2, result3) matching reference().

Sharding: row-parallel attention across 8 NeuronCores. Each core owns 512
query rows (i), holds the full Wh (all j), computes its rows of every
attention matrix, and the h matrix is exchanged via an on-device AllGather
between GAT layer 1 and layer 2.

Key structure per core:
  - scores are built directly in transposed orientation [j on partitions,
    i free] so the attn @ Wh contraction (over j) needs no transposes.
  - softmax max-subtraction is dropped: exp(-max) cancels in the
    (P @ Wh) / (P @ 1) ratio, and scores here are bounded (~+-8) so exp is
    safe in f32. The denominator comes from a ones column in the stationary
    matmul operand.
  - exp(leaky_relu(z)): z = f1_i + f2_j via one DVE tensor_scalar add
    (per-partition scalar = f2), leaky relu via one scalar_tensor_tensor
    max(0.2*z, z), exp on ACT, mask multiply on DVE (mask prepped host-side
    as bf16 adj^T shards; adj values are 0/1).
"""

import os

os.environ.setdefault("MYCRO_LOCAL_CACHE", "1")

import numpy as np
import ml_dtypes

import concourse.bass as bass
import concourse.tile as tile
from concourse import mybir
from concourse.bass_utils import run_bass_kernel_spmd

F32 = mybir.dt.float32
BF16 = mybir.dt.bfloat16
AF = mybir.ActivationFunctionType
ALU = mybir.AluOpType

N_CORES = 8
N = 4096
NFEAT = 512
NHID = 64
NCLASS = 16
DEG_MAX = 128
NHEADS = 8
R = N // N_CORES          # 512 rows per core
NJC = N // 128            # 32 j-chunks
NNB = N // 128            # 32 n-blocks
NCC = NFEAT // 128        # 4 feature chunks

MAX_WAITS = 1


class SplitDrainTC(tile.TileContext):
    """TileContext whose final drain splits sem-waits into <=MAX_WAITS chunks.

    The walrus build here rejects instructions with more than one sync wait
    ("Too many sync wait commands"); the stock kernel-tail drain waits on
    every outstanding logical processor at once.
    """

    def _drain_and_barrier(self, tick_clock, wait_clock):
        from concourse.vector_clock import ScopedClock

        nc = self.nc
        probe = nc.sync.nop()
        wait_clock.add_sem_waits(
            probe.ins, ScopedClock({None: tick_clock.global_clock})
        )
        si = probe.ins.sync_info
        waits = list(si.on_wait) if si is not None and si.on_wait else []
        if len(waits) > MAX_WAITS:
            chunks = [waits[i:i + MAX_WAITS]
                      for i in range(0, len(waits), MAX_WAITS)]
            probe.ins.sync_info = mybir.SyncInfo(on_wait=chunks[0], on_update=[])
            for ch in chunks[1:]:
                extra = nc.sync.nop()
                extra.ins.sync_info = mybir.SyncInfo(on_wait=ch, on_update=[])
        nc.sync.drain()
        nc.all_engine_barrier()
        assert self.sems is not None
        popped = nc._tile_sem_poison_stack.pop()
        assert popped is self._sem_poison
        nc.clear_and_free_semaphores(list(self.sems.allocated().values()))
        nc.all_engine_barrier()


def _split_multi_waits(nc):
    """Rewrite any instruction carrying >MAX_WAITS sem waits into a chain of
    single-wait nops on the same engine followed by the instruction."""
    for bbw in nc.main_func.blocks:
        insts = list(bbw.instructions)
        heavy = []
        for ins in insts:
            si = ins.sync_info
            if si is not None and si.on_wait and len(si.on_wait) > MAX_WAITS:
                heavy.append(ins)
        if not heavy:
            continue
        # Pre-create nops (they append to the current bb; we pull them off the
        # tail of whatever block they landed in and re-place them).
        need = sum(len(i.sync_info.on_wait) - MAX_WAITS for i in heavy)
        spare = []
        for _ in range(need):
            spare.append(nc.sync.nop().ins)
        cur = nc.cur_bb.bb
        cur_list = [x for x in cur.instructions if x.name not in
                    {s.name for s in spare}]
        cur.instructions = cur_list
        out = []
        for ins in insts:
            si = ins.sync_info
            if ins in heavy:
                waits = list(si.on_wait)
                extra, keep = waits[:-MAX_WAITS], waits[-MAX_WAITS:]
                for w in extra:
                    nop = spare.pop()
                    nop.engine = ins.engine
                    nop.sync_info = mybir.SyncInfo(on_wait=[w], on_update=[])
                    out.append(nop)
                ins.sync_info = mybir.SyncInfo(
                    on_wait=keep,
                    on_update=list(si.on_update) if si.on_update else [],
                )
            out.append(ins)
        bbw.instructions = out


def build_nc():
    nc = bass.Bass(num_devices=N_CORES)

    # ---- I/O ----
    xT = nc.dram_tensor("xT", [NFEAT, N], BF16, kind="ExternalInput")
    xTs = nc.dram_tensor("xTs", [NFEAT, R], F32, kind="ExternalInput")
    adjT = nc.dram_tensor("adjT", [N, R], BF16, kind="ExternalInput")
    rh1 = nc.dram_tensor("rh1", [NHEADS, NFEAT, NHID + 1], BF16,
                         kind="ExternalInput")
    v1all = nc.dram_tensor("v1all", [NFEAT, NHEADS], F32, kind="ExternalInput")
    rh2 = nc.dram_tensor("rh2", [NFEAT, 82], BF16, kind="ExternalInput")
    v1ab = nc.dram_tensor("v1ab", [NFEAT, 2], BF16, kind="ExternalInput")
    wd = nc.dram_tensor("wd", [NFEAT, NHID], F32, kind="ExternalInput")
    bdown = nc.dram_tensor("bdown", [NHID, 1], F32, kind="ExternalInput")
    rhs3 = nc.dram_tensor("rhs3", [NHID, 1 + DEG_MAX], F32,
                          kind="ExternalInput")
    bias3 = nc.dram_tensor("bias3", [1, 1 + DEG_MAX], F32,
                           kind="ExternalInput")

    res1 = nc.dram_tensor("res1", [R, NCLASS], F32, kind="ExternalOutput")
    res2 = nc.dram_tensor("res2", [R, 1], F32, kind="ExternalOutput")
    res3 = nc.dram_tensor("res3", [R, DEG_MAX], F32, kind="ExternalOutput")

    with SplitDrainTC(nc) as tc:
        _build_body(nc, tc, xT, xTs, adjT, rh1, v1all, rh2, v1ab, wd, bdown,
                    rhs3, bias3, res1, res2, res3)
    _split_multi_waits(nc)
    return nc


def _build_body(nc, tc, xT, xTs, adjT, rh1, v1all, rh2, v1ab, wd, bdown,
                rhs3, bias3, res1, res2, res3):
    from contextlib import ExitStack

    ctx = ExitStack()
    with ctx:
        persist = ctx.enter_context(tc.tile_pool(name="persist", bufs=1))
        dram = ctx.enter_context(tc.tile_pool(name="dram", bufs=1,
                                              space="DRAM"))

        # ---------- persistent loads ----------
        mask_all = persist.tile([128, NJC, R], BF16, tag="mask")
        nc.sync.dma_start(mask_all[:],
                          adjT.rearrange("(j p) i -> p j i", p=128))
        xTs_all = persist.tile([128, NCC, R], F32, tag="xts")
        nc.sync.dma_start(xTs_all[:],
                          xTs.rearrange("(c p) i -> p c i", p=128))
        rh1_sb = persist.tile([128, NCC, NHEADS, NHID + 1], BF16, tag="rh1")
        for c in range(NCC):
            nc.sync.dma_start(
                rh1_sb[:, c, :, :],
                rh1[:, c * 128:(c + 1) * 128, :].rearrange("h p f -> p h f"))
        v1_sb = persist.tile([128, NCC, NHEADS], F32, tag="v1")
        nc.sync.dma_start(v1_sb[:],
                          v1all.rearrange("(c p) h -> p c h", p=128))
        rh2_sb = persist.tile([128, NCC, 82], BF16, tag="rh2")
        nc.sync.dma_start(rh2_sb[:],
                          rh2.rearrange("(c p) f -> p c f", p=128))
        v1ab_sb = persist.tile([128, NCC, 2], BF16, tag="v1ab")
        nc.sync.dma_start(v1ab_sb[:],
                          v1ab.rearrange("(c p) s -> p c s", p=128))
        wd_sb = persist.tile([128, NCC, NHID], F32, tag="wd")
        nc.sync.dma_start(wd_sb[:], wd.rearrange("(c p) f -> p c f", p=128))
        bdown_sb = persist.tile([NHID, 1], F32, tag="bdown")
        nc.sync.dma_start(bdown_sb[:], bdown[:])
        rhs3_sb = persist.tile([NHID, 1 + DEG_MAX], F32, tag="rhs3")
        nc.sync.dma_start(rhs3_sb[:], rhs3[:])
        b3row = persist.tile([1, 1 + DEG_MAX], F32, tag="b3row")
        nc.sync.dma_start(b3row[:], bias3[:])

        ones1 = persist.tile([1, 128], F32, tag="ones1")
        nc.vector.memset(ones1[:], 1.0)
        ident = persist.tile([128, 128], F32, tag="ident")
        from concourse.masks import make_identity
        make_identity(nc, ident[:])

        # h^T for this core's own columns (512 feats x R), written per head.
        hTs_all = persist.tile([128, NCC, R], BF16, tag="hts")
        # original_x^T [64, R]
        origxT = persist.tile([NHID, R], F32, tag="origx")
        # bias3 broadcast [128, 129]
        b3_sb = persist.tile([128, 1 + DEG_MAX], F32, tag="b3")

        # AG bounce buffers
        ag_in = dram.tile([R, R], BF16)
        ag_out = dram.tile([N, R], BF16, addr_space="Shared")

        # ---------- phase A/B: layer 1 ----------
        with tc.tile_pool(name="xtpool", bufs=1) as xtpool, \
             tc.tile_pool(name="l1work", bufs=3) as work, \
             tc.tile_pool(name="swhpool", bufs=2) as swhpool, \
             tc.tile_pool(name="pwh", bufs=2, space="PSUM") as pwh, \
             tc.tile_pool(name="pat", bufs=2, space="PSUM") as pat, \
             tc.tile_pool(name="pbc", bufs=2, space="PSUM") as pbc:

            xT_all = xtpool.tile([128, NCC, N], BF16, tag="xt")
            nc.sync.dma_start(xT_all[:],
                              xT.rearrange("(c p) n -> p c n", p=128))

            # original_x^T = W_down^T @ x^T(shard cols) + b_down
            ps_ox = pat.tile([NHID, R], F32, tag="at")
            for c in range(NCC):
                nc.tensor.matmul(ps_ox[:], wd_sb[:, c, :], xTs_all[:, c, :],
                                 start=(c == 0), stop=(c == NCC - 1))
            nc.vector.tensor_scalar_add(origxT[:], ps_ox[:], bdown_sb[:])

            # bias3 broadcast to 128 partitions
            ps_b3 = pbc.tile([128, 1 + DEG_MAX], F32, tag="b3f1")
            nc.tensor.matmul(ps_b3[:], ones1[:], b3row[:],
                             start=True, stop=True)
            nc.vector.tensor_copy(b3_sb[:], ps_b3[:])


            # Wh sweeps, 4 heads at a time
            # Swh4 layout per head hh in group: base b=66*hh:
            #   col b: ones, b+1..b+64: Wh, b+65: f2
            swh = {}
            f2st = {}
            for g in range(2):
                swh4 = swhpool.tile([128, NNB, 4 * 65], BF16, tag="swh")
                swh[g] = swh4
                f2t = swhpool.tile([128, NNB, 4], F32, tag="f2st")
                f2st[g] = f2t
                sw4 = swh4.rearrange("p n (h f) -> p n h f", h=4)
                nc.vector.memset(sw4[:, :, :, 64:65], 1.0)
                for nb in range(NNB):
                    ps = pwh.tile([128, 4 * 65], F32, tag="wh")
                    for c in range(NCC):
                        rhs4 = rh1_sb[:, c, 4 * g:4 * g + 4, :].rearrange(
                            "p h f -> p (h f)")
                        nc.tensor.matmul(
                            ps[:], xT_all[:, c, nb * 128:(nb + 1) * 128],
                            rhs4, start=(c == 0), stop=(c == NCC - 1))
                    psr = ps.rearrange("p (h f) -> p h f", h=4)
                    nc.vector.tensor_copy(sw4[:, nb, :, 0:64],
                                          psr[:, :, 0:64])
                    nc.vector.tensor_copy(f2t[:, nb, :], psr[:, :, 64])

            # per-head attention
            for h in range(NHEADS):
                g, hh = divmod(h, 4)
                b = 65 * hh
                swh4 = swh[g]
                # broadcast f1[h] to [128, R] bf16
                ps_f1h = pbc.tile([1, R], F32, tag="b3f1")
                for c in range(NCC):
                    nc.tensor.matmul(ps_f1h[:], v1_sb[:, c, h:h + 1],
                                     xTs_all[:, c, :],
                                     start=(c == 0), stop=(c == NCC - 1))
                f1one = work.tile([1, R], F32, tag="f1one", bufs=2)
                nc.vector.tensor_copy(f1one[:], ps_f1h[:])
                ps_bc = pbc.tile([128, R], F32, tag="bcrb")
                nc.tensor.matmul(ps_bc[:], ones1[:], f1one[:],
                                 start=True, stop=True)
                F1 = work.tile([128, R], BF16, tag="F1", bufs=2)
                nc.vector.tensor_copy(F1[:], ps_bc[:])

                ps_at = pat.tile([NHID + 1, R], F32, tag="at")
                for jc in range(NJC):
                    z = work.tile([128, R], BF16, tag="z")
                    nc.vector.tensor_scalar_add(
                        z[:], F1[:], f2st[g][:, jc, hh:hh + 1])
                    l = work.tile([128, R], BF16, tag="l")
                    nc.vector.scalar_tensor_tensor(
                        l[:], z[:], 0.2, z[:], ALU.mult, ALU.max)
                    e = work.tile([128, R], BF16, tag="e")
                    nc.scalar.activation(e[:], l[:], AF.Exp)
                    pt = work.tile([128, R], BF16, tag="pt")
                    nc.vector.tensor_tensor(
                        pt[:], e[:], mask_all[:, jc, :], op=ALU.mult)
                    nc.tensor.matmul(ps_at[:], swh4[:, jc, b:b + 65], pt[:],
                                     start=(jc == 0), stop=(jc == NJC - 1))

                # normalize + elu -> hTs_all
                rcp = work.tile([1, R], F32, tag="rcp", bufs=2)
                rscr = work.tile([1, R], F32, tag="rscr", bufs=2)
                nc.scalar.activation(rscr[:], ps_at[NHID:NHID + 1, :], AF.Ln)
                nc.scalar.activation(rcp[:], rscr[:], AF.Exp, scale=-1.0)
                ps_rb = pbc.tile([NHID, R], F32, tag="bcrb")
                nc.tensor.matmul(ps_rb[:], ones1[0:1, 0:NHID], rcp[:],
                                 start=True, stop=True)
                rb = work.tile([NHID, R], F32, tag="rb_sb", bufs=1)
                nc.vector.tensor_copy(rb[:], ps_rb[:])
                hraw = work.tile([NHID, R], F32, tag="hraw", bufs=1)
                nc.vector.tensor_tensor(hraw[:], ps_at[0:NHID, :], rb[:],
                                        op=ALU.mult)
                mn = work.tile([NHID, R], F32, tag="mn", bufs=1)
                nc.vector.tensor_scalar_min(mn[:], hraw[:], 0.0)
                ex = work.tile([NHID, R], F32, tag="ex", bufs=1)
                nc.scalar.activation(ex[:], mn[:], AF.Exp)
                px = work.tile([NHID, R], F32, tag="px", bufs=1)
                nc.vector.tensor_scalar_max(px[:], hraw[:], 0.0)
                dst = hTs_all[64 * (h % 2):64 * (h % 2) + 64, h // 2, :]
                nc.vector.scalar_tensor_tensor(
                    dst, px[:], -1.0, ex[:], ALU.add, ALU.add)

        # ---------- phase C: allgather h^T ----------
        nc.gpsimd.dma_start(ag_in.rearrange("(c p) i -> p c i", p=128),
                            hTs_all[:])
        nc.gpsimd.collective_compute(
            "AllGather", ALU.bypass,
            replica_groups=[list(range(N_CORES))],
            ins=[ag_in[:].opt()], outs=[ag_out[:].opt()])

        # ---------- phase D: layer 2 ----------
        with tc.tile_pool(name="htpool", bufs=1) as htpool, \
             tc.tile_pool(name="l2work", bufs=3) as work, \
             tc.tile_pool(name="s2pool", bufs=1) as s2pool, \
             tc.tile_pool(name="pw2", bufs=2, space="PSUM") as pw2, \
             tc.tile_pool(name="pat2", bufs=2, space="PSUM") as pat2, \
             tc.tile_pool(name="pbc2", bufs=2, space="PSUM") as pbc2:

            hT_all = htpool.tile([128, NCC, N], BF16, tag="ht")
            src = ag_out.rearrange("(r cc p) i -> cc p r i", r=N_CORES,
                                   cc=NCC, p=128)
            for cc in range(NCC):
                nc.sync.dma_start(
                    hT_all[:, cc, :].rearrange("p (r i) -> p r i", r=N_CORES),
                    src[cc])

            # Wh sweep for both sublayers
            # S2 layout: 0 ones_a, 1:17 Wh1, 17 f2a, 18 ones_b, 19:83 Wh2,
            # 83 f2b
            s2 = s2pool.tile([128, NNB, 98], BF16, tag="s2")
            f2ab = s2pool.tile([128, NNB, 2], F32, tag="f2ab")
            nc.vector.memset(s2[:, :, 16:32], 0.0)
            nc.vector.memset(s2[:, :, 32:33], 1.0)
            nc.vector.memset(s2[:, :, 97:98], 1.0)
            for nb in range(NNB):
                ps = pw2.tile([128, 82], F32, tag="w2")
                for c in range(NCC):
                    nc.tensor.matmul(
                        ps[:], hT_all[:, c, nb * 128:(nb + 1) * 128],
                        rh2_sb[:, c, :], start=(c == 0), stop=(c == NCC - 1))
                nc.vector.tensor_copy(s2[:, nb, 0:16], ps[:, 0:16])
                nc.vector.tensor_copy(s2[:, nb, 33:97], ps[:, 17:81])
                nc.vector.tensor_copy(f2ab[:, nb, 0:1], ps[:, 16:17])
                nc.vector.tensor_copy(f2ab[:, nb, 1:2], ps[:, 81:82])


            ps_res = {}
            for s, (base, m) in enumerate(((0, 33), (33, 65))):
                ps_f = pbc2.tile([1, R], F32, tag="bc2")
                for c in range(NCC):
                    nc.tensor.matmul(ps_f[:], v1ab_sb[:, c, s:s + 1],
                                     hTs_all[:, c, :],
                                     start=(c == 0), stop=(c == NCC - 1))
                fone = work.tile([1, R], F32, tag="fone", bufs=2)
                nc.vector.tensor_copy(fone[:], ps_f[:])
                ps_bc = pbc2.tile([128, R], F32, tag="bc2")
                nc.tensor.matmul(ps_bc[:], ones1[:], fone[:],
                                 start=True, stop=True)
                F1 = work.tile([128, R], BF16, tag="F1b", bufs=2)
                nc.vector.tensor_copy(F1[:], ps_bc[:])
                ps_at = pat2.tile([NHID + 1, R], F32, tag=f"at{s}",
                                  bufs=1)
                ps_res[s] = ps_at
                for jc in range(NJC):
                    z = work.tile([128, R], BF16, tag="z2")
                    nc.vector.tensor_scalar_add(
                        z[:], F1[:], f2ab[:, jc, s:s + 1])
                    l = work.tile([128, R], BF16, tag="l2")
                    nc.vector.scalar_tensor_tensor(
                        l[:], z[:], 0.2, z[:], ALU.mult, ALU.max)
                    e = work.tile([128, R], BF16, tag="e2")
                    nc.scalar.activation(e[:], l[:], AF.Exp)
                    pt = work.tile([128, R], BF16, tag="pt2")
                    nc.vector.tensor_tensor(
                        pt[:], e[:], mask_all[:, jc, :], op=ALU.mult)
                    nc.tensor.matmul(
                        ps_at[0:m, :], s2[:, jc, base:base + m],
                        pt[:], start=(jc == 0), stop=(jc == NJC - 1))

            # classify sublayer (a): normalize, elu, transpose, log_softmax
            ps_a = ps_res[0]
            rcp = work.tile([1, R], F32, tag="rcpa", bufs=2)
            rscra = work.tile([1, R], F32, tag="rscra", bufs=2)
            nc.scalar.activation(rscra[:], ps_a[32:33, :], AF.Ln)
            nc.scalar.activation(rcp[:], rscra[:], AF.Exp, scale=-1.0)
            ps_rb = pbc2.tile([NCLASS, R], F32, tag="bc2")
            nc.tensor.matmul(ps_rb[:], ones1[0:1, 0:NCLASS], rcp[:],
                             start=True, stop=True)
            rb = work.tile([NCLASS, R], F32, tag="rba_sb", bufs=1)
            nc.vector.tensor_copy(rb[:], ps_rb[:])
            craw = work.tile([NCLASS, R], F32, tag="craw", bufs=1)
            nc.vector.tensor_tensor(craw[:], ps_a[0:NCLASS, :], rb[:],
                                    op=ALU.mult)
            # elu
            mn = work.tile([NCLASS, R], F32, tag="mna", bufs=1)
            nc.vector.tensor_scalar_min(mn[:], craw[:], 0.0)
            ex = work.tile([NCLASS, R], F32, tag="exa", bufs=1)
            nc.scalar.activation(ex[:], mn[:], AF.Exp)
            px = work.tile([NCLASS, R], F32, tag="pxa", bufs=1)
            nc.vector.tensor_scalar_max(px[:], craw[:], 0.0)
            clsT = work.tile([NCLASS, R], F32, tag="clsT", bufs=1)
            nc.vector.scalar_tensor_tensor(clsT[:], px[:], -1.0, ex[:],
                                           ALU.add, ALU.add)

            # z sublayer (b): normalize + original_x
            ps_b = ps_res[1]
            rcpb = work.tile([1, R], F32, tag="rcpb", bufs=2)
            rscrb = work.tile([1, R], F32, tag="rscrb", bufs=2)
            nc.scalar.activation(rscrb[:], ps_b[64:65, :], AF.Ln)
            nc.scalar.activation(rcpb[:], rscrb[:], AF.Exp, scale=-1.0)
            ps_rbb = pbc2.tile([NHID, R], F32, tag="bc2")
            nc.tensor.matmul(ps_rbb[:], ones1[0:1, 0:NHID], rcpb[:],
                             start=True, stop=True)
            rbb = work.tile([NHID, R], F32, tag="rbb_sb", bufs=1)
            nc.vector.tensor_copy(rbb[:], ps_rbb[:])
            zraw = work.tile([NHID, R], F32, tag="zraw", bufs=1)
            nc.vector.tensor_tensor(zraw[:], ps_b[0:NHID, :], rbb[:],
                                    op=ALU.mult)
            zT = work.tile([NHID, R], F32, tag="zT", bufs=1)
            nc.vector.tensor_tensor(zT[:], zraw[:], origxT[:], op=ALU.add)

            # outputs
            for ib in range(R // 128):
                sl = slice(ib * 128, ib * 128 + 128)
                # result1: transpose classify block then log_softmax
                ps_t = pbc2.tile([128, NCLASS], F32, tag="tr", bufs=1)
                nc.tensor.transpose(ps_t[:], clsT[:, sl],
                                    ident[0:NCLASS, 0:NCLASS])
                rmax = work.tile([128, 1], F32, tag="rmax", bufs=2)
                nc.vector.tensor_reduce(rmax[:], ps_t[:],
                                        axis=mybir.AxisListType.X, op=ALU.max)
                u = work.tile([128, NCLASS], F32, tag="u", bufs=1)
                nc.vector.tensor_scalar_sub(u[:], ps_t[:], rmax[:])
                e2 = work.tile([128, NCLASS], F32, tag="e2s", bufs=1)
                nc.scalar.activation(e2[:], u[:], AF.Exp)
                ssum = work.tile([128, 1], F32, tag="ssum", bufs=2)
                nc.vector.tensor_reduce(ssum[:], e2[:],
                                        axis=mybir.AxisListType.X, op=ALU.add)
                lg = work.tile([128, 1], F32, tag="lg", bufs=2)
                nc.scalar.activation(lg[:], ssum[:], AF.Ln)
                r1t = work.tile([128, NCLASS], F32, tag="r1t", bufs=1)
                nc.vector.tensor_scalar_sub(r1t[:], u[:], lg[:])
                nc.sync.dma_start(res1[sl, :], r1t[:])

                # result2/3: z @ [w_deg | W_deg3] + bias
                ps_o = pw2.tile([128, 1 + DEG_MAX], F32, tag="w2")
                nc.tensor.matmul(ps_o[:], zT[:, sl], rhs3_sb[:],
                                 start=True, stop=True)
                r23 = work.tile([128, 1 + DEG_MAX], F32, tag="r23", bufs=1)
                nc.vector.tensor_tensor(r23[:], ps_o[:], b3_sb[:], op=ALU.add)
                nc.sync.dma_start(res2[sl, :], r23[:, 0:1])
                nc.sync.dma_start(res3[sl, :], r23[:, 1:1 + DEG_MAX])


_NC_CACHE = None


def _get_nc():
    global _NC_CACHE
    if _NC_CACHE is None:
        _NC_CACHE = build_nc()
    return _NC_CACHE


def kernel(x, adj, W_att, a_att, W_out1, a_out1, W_out2, a_out2,
           W_down, b_down, w_deg, b_deg, W_deg3, b_deg3):
    x = np.asarray(x, dtype=np.float32)
    adj = np.asarray(adj)
    W_att = np.asarray(W_att, dtype=np.float32)
    a_att = np.asarray(a_att, dtype=np.float32)

    xT = np.ascontiguousarray(x.T)                       # [512, 4096]
    xT_bf = xT.astype(ml_dtypes.bfloat16)
    mask_bf = (adj > 0).astype(ml_dtypes.bfloat16)       # [4096, 4096]

    # layer-1 per-head rhs: [W_h | W_h @ a_hi], and v1 = W_h @ a_lo
    rh1 = np.empty((NHEADS, NFEAT, NHID + 1), np.float32)  # cast below
    v1all = np.empty((NFEAT, NHEADS), np.float32)
    for h in range(NHEADS):
        rh1[h, :, :NHID] = W_att[h]
        rh1[h, :, NHID] = W_att[h] @ a_att[h, NHID:]
        v1all[:, h] = W_att[h] @ a_att[h, :NHID]

    W_out1 = np.asarray(W_out1, dtype=np.float32)
    W_out2 = np.asarray(W_out2, dtype=np.float32)
    a_out1 = np.asarray(a_out1, dtype=np.float32)
    a_out2 = np.asarray(a_out2, dtype=np.float32)
    rh2 = np.empty((NFEAT, 82), np.float32)
    rh2[:, 0:16] = W_out1
    rh2[:, 16] = W_out1 @ a_out1[NCLASS:]
    rh2[:, 17:81] = W_out2
    rh2[:, 81] = W_out2 @ a_out2[NHID:]
    v1ab = np.stack([W_out1 @ a_out1[:NCLASS],
                     W_out2 @ a_out2[:NHID]], axis=1)    # [512, 2]
    rh1_bf = rh1.astype(ml_dtypes.bfloat16)
    rh2_bf = rh2.astype(ml_dtypes.bfloat16)
    v1ab_bf = v1ab.astype(ml_dtypes.bfloat16)

    wd = np.asarray(W_down, dtype=np.float32)
    bdown = np.asarray(b_down, dtype=np.float32).reshape(NHID, 1)
    rhs3 = np.concatenate([np.asarray(w_deg, np.float32),
                           np.asarray(W_deg3, np.float32)], axis=1)
    bias3 = np.concatenate([np.asarray(b_deg, np.float32),
                            np.asarray(b_deg3, np.float32)]).reshape(1, -1)

    in_maps = []
    for c in range(N_CORES):
        i0 = c * R
        in_maps.append({
            "xT": xT_bf,
            "xTs": np.ascontiguousarray(xT[:, i0:i0 + R]),
            "adjT": np.ascontiguousarray(mask_bf[i0:i0 + R, :].T),
            "rh1": rh1_bf,
            "v1all": v1all,
            "rh2": rh2_bf,
            "v1ab": v1ab_bf,
            "wd": wd,
            "bdown": bdown,
            "rhs3": rhs3,
            "bias3": bias3,
        })

    nc = _get_nc()
    res = run_bass_kernel_spmd(nc, in_maps, core_ids=list(range(N_CORES)))
    r1 = np.concatenate([res.results[c]["res1"] for c in range(N_CORES)], 0)
    r2 = np.concatenate([res.results[c]["res2"] for c in range(N_CORES)],
                        0)[:, 0]
    r3 = np.concatenate([res.results[c]["res3"] for c in range(N_CORES)], 0)
    return r1, r2, r3
